# revision 24
# baseline (speedup 1.0000x reference)
"""Diagonal SSM kernel for 8 Trainium2 NeuronCores.

Math (per batch element b, sharded one per core):
    alpha = sigmoid(u @ Wa.T + ba)          (S, N)
    Bu    = u @ Wb.T + bb                   (S, N)
    x_t   = alpha_t * x_{t-1} + Bu_t        (scan over S)
    y     = xs @ C.T + u @ Dm.T             (S, D)

Device strategy (per core):
  - u (S, D) fp32 is DMA'd in naturally (HWDGE plain copies), downcast to
    bf16 on DVE/ACT, transposed on the TensorEngine (bf16 transpose-mode
    matmul against an identity, 1 cyc/row) into PSUM, and copied out both
    as bf16 uT [D x S] tiles (for GEMM-B) and as an fp8e4 copy (for
    GEMM-A). No SWDGE, no DRAM scratch: SWDGE cast-DMAs starve the HWDGE
    queues on the shared SDMA engines.
  - GEMM-A in fp8 DoubleRow (2 k-tiles per matmul, half the PE cycles):
    psum[n, s-chunk] = sum_d Wab8T[d, n-tile] . uT8[d, s-chunk], with the
    weights pre-scaled by 64 on host (keeps them out of the fp8 subnormal
    range) and the 1/64 rescale folded into the ScalarE activation that
    applies sigmoid(+ba) / identity(+bb) straight out of PSUM. The
    alpha/Bu path only contributes ~3% of the output magnitude, so fp8
    error here is strongly attenuated.
  - Recurrence: native VectorE tensor_tensor_scan (op0=mult, op1=add,
    fp32 internal state) along the free dim, chunk-chained via a
    per-partition initial value.
  - GEMM-B (bf16 - it dominates the output magnitude): y[s-tile, d] =
    xsT.T @ CT + uT.T @ DmT accumulated in PSUM, copied to SBUF
    (DVE/ACT alternating) and DMA'd out as fp32.
  - Emission is software-pipelined: ingest for chunk sc+2 and GEMM-B for
    chunk sc-1 are emitted around GEMM-A(sc) so the PE queue never heads
    into a matmul whose scan dependency hasn't cleared yet.

Params are pre-packed on host (transposed, fp8/bf16) - standard weight
packing. The full u tensor is read on device in fp32.
"""

import numpy as np
import ml_dtypes

B, S, D, N = 8, 4096, 1024, 256
NCORES = 8
KT = D // 128          # 8 contraction tiles
SC = 512               # s-chunk (matmul free dim / PSUM bank / ingest chunk)
NSC = S // SC          # 8 s-chunks
WAB_SCALE = 64.0       # fp8 weight pre-scale for GEMM-A

_CACHE = {}
LAST_RESULTS = None    # test harness reads profiling info from here


def _build_program():
    import concourse.mybir as mybir
    import concourse.tile as tile
    from concourse import bacc
    from concourse.masks import make_identity

    fp32 = mybir.dt.float32
    bf16 = mybir.dt.bfloat16
    fp8 = mybir.dt.float8e4
    AF = mybir.ActivationFunctionType
    OP = mybir.AluOpType
    DR = mybir.MatmulPerfMode.DoubleRow

    nc = bacc.Bacc(
        "TRN2",
        target_bir_lowering=False,
        debug=False,
        enable_asserts=False,
        num_devices=NCORES,
    )

    u = nc.dram_tensor("u", [S, D], fp32, kind="ExternalInput").ap()
    wab8 = nc.dram_tensor("wab8", [128, KT, 2 * N], fp8, kind="ExternalInput").ap()
    bias = nc.dram_tensor("bias", [128, 4], fp32, kind="ExternalInput").ap()
    ct = nc.dram_tensor("ct", [N, D], bf16, kind="ExternalInput").ap()
    dmt = nc.dram_tensor("dmt", [D, D], bf16, kind="ExternalInput").ap()
    y = nc.dram_tensor("y", [S, D], fp32, kind="ExternalOutput").ap()

    with tile.TileContext(nc) as tc:
        with (
            tc.tile_pool(name="consts", bufs=1) as consts,
            tc.tile_pool(name="data", bufs=1) as data,
            tc.tile_pool(name="unat", bufs=4) as unat,
            tc.tile_pool(name="ab", bufs=3) as abpool,
            tc.tile_pool(name="xs", bufs=3) as xspool,
            tc.tile_pool(name="psA", bufs=2, space="PSUM") as psA,
            tc.tile_pool(name="psB", bufs=3, space="PSUM") as psB,
            tc.tile_pool(name="psT", bufs=3, space="PSUM") as psT,
            tc.tile_pool(name="ypool", bufs=4) as ypool,
        ):
            # ---- param tiles (loads emitted in startup-criticality order below) ----
            wab8_sb = consts.tile([128, KT, 2 * N], fp8, name="wab8_sb")
            ct_sb = [consts.tile([128, D], bf16, name=f"ct{h}") for h in range(2)]
            dmt_sb = [consts.tile([128, D], bf16, name=f"dmt{k}") for k in range(KT)]
            bias_sb = consts.tile([128, 4], fp32, name="bias_sb")
            ident16_sb = consts.tile([128, 128], bf16, name="ident16_sb")

            def load_params_early():
                # built on the (otherwise idle) GpSimd engine: no DMA in the
                # critical startup chain
                make_identity(nc, ident16_sb[:])

            def load_params_mid():
                # needed by gemm_a(0): weights + activation biases
                nc.sync.dma_start(out=wab8_sb[:], in_=wab8[:])
                nc.sync.dma_start(out=bias_sb[:], in_=bias[:])

            def load_params_late():
                # needed by gemm_b(0), which runs after gemm_a(1)
                for h in range(2):
                    nc.sync.dma_start(out=ct_sb[h][:], in_=ct[h * 128:(h + 1) * 128, :])
                for k in range(KT):
                    nc.sync.dma_start(out=dmt_sb[k][:], in_=dmt[k * 128:(k + 1) * 128, :])

            uT = [data.tile([128, S], bf16, name=f"uT{k}") for k in range(KT)]
            uT8 = data.tile([128, KT, S], fp8, name="uT8")

            def ingest(sc):
                """Load 4 s-tiles of u (fp32), downcast to bf16, PE-transpose
                each 128x128 block into PSUM, copy into uT (bf16) and
                uT8 (fp8, for the DoubleRow GEMM-A)."""
                ssl = slice(sc * SC, (sc + 1) * SC)
                ut_tiles = []
                for t in range(4):
                    st = sc * 4 + t
                    un = unat.tile([128, D], fp32, name="unat", tag="unat")
                    nc.sync.dma_start(out=un[:], in_=u[st * 128:(st + 1) * 128, :])
                    un16 = unat.tile([128, D], bf16, name="un16", tag="un16", bufs=8)
                    nc.vector.tensor_copy(un16[:], un[:])
                    ut_tiles.append(un16)
                for k in range(KT):
                    ps = psT.tile([128, SC], bf16, name="pst", tag="pst")
                    for t in range(4):
                        nc.tensor.transpose(
                            ps[:, t * 128:(t + 1) * 128],
                            ut_tiles[t][:, k * 128:(k + 1) * 128],
                            ident16_sb[:],
                        )
                    nc.scalar.copy(uT[k][:, ssl], ps[:])
                    nc.vector.tensor_copy(uT8[:, k, ssl], ps[:])

            def gemm_a(sc):
                """fp8 DoubleRow GEMM for alpha/Bu; the 1/WAB_SCALE rescale is
                folded into the ScalarE activation. Returns the chunk tiles
                [alpha_h0, alpha_h1, bu_h0, bu_h1]."""
                ssl = slice(sc * SC, (sc + 1) * SC)
                out_tiles = []
                for nt in range(4):
                    ps = psA.tile([128, SC], fp32, name="psa", tag="psa")
                    for kp in range(KT // 2):
                        nc.tensor.matmul(
                            ps[:],
                            wab8_sb[:, 2 * kp:2 * kp + 2, nt * 128:(nt + 1) * 128],
                            uT8[:, 2 * kp:2 * kp + 2, ssl],
                            start=(kp == 0),
                            stop=(kp == KT // 2 - 1),
                            perf_mode=DR,
                        )
                    o = abpool.tile([128, SC], bf16, name=f"ab{nt}", tag=f"ab{nt}")
                    nc.scalar.activation(
                        o[:], ps[:],
                        AF.Sigmoid if nt < 2 else AF.Identity,
                        bias=bias_sb[:, nt:nt + 1],
                        scale=1.0 / WAB_SCALE,
                    )
                    out_tiles.append(o)
                return out_tiles

            def scan(sc, ab_tiles, prev_xs):
                """Returns this chunk's xs tiles (one per 128-channel half)."""
                xs_tiles = []
                for h in range(2):
                    o = xspool.tile([128, SC], bf16, name=f"xs{h}", tag=f"xs{h}")
                    init = 0.0 if prev_xs is None else prev_xs[h][:, SC - 1:SC]
                    nc.vector.tensor_tensor_scan(
                        o[:],
                        ab_tiles[h][:],
                        ab_tiles[2 + h][:],
                        init,
                        op0=OP.mult,
                        op1=OP.add,
                    )
                    xs_tiles.append(o)
                return xs_tiles

            def gemm_b(sc, xs_tiles):
                for t in range(4):
                    st = sc * 4 + t
                    stsl = slice(st * 128, (st + 1) * 128)
                    tsl = slice(t * 128, (t + 1) * 128)
                    ytile = ypool.tile([128, D], fp32, name="ytile", tag="ytile")
                    for dc in range(2):
                        dsl = slice(dc * SC, (dc + 1) * SC)
                        ps = psB.tile([128, SC], fp32, name="psb", tag="psb")
                        nc.tensor.matmul(ps[:], xs_tiles[0][:, tsl], ct_sb[0][:, dsl],
                                         start=True, stop=False)
                        nc.tensor.matmul(ps[:], xs_tiles[1][:, tsl], ct_sb[1][:, dsl],
                                         start=False, stop=False)
                        for k in range(KT):
                            nc.tensor.matmul(ps[:], uT[k][:, stsl], dmt_sb[k][:, dsl],
                                             start=False, stop=(k == KT - 1))
                        nc.vector.tensor_copy(ytile[:, dsl], ps[:])
                    nc.sync.dma_start(out=y[stsl, :], in_=ytile[:])

            # ---- software-pipelined emission ----
            load_params_early()
            ingest(0)
            load_params_mid()
            ingest(1)
            load_params_late()
            ingest(2)
            ab = gemm_a(0)
            xs_prev = scan(0, ab, None)
            for sc in range(1, NSC):
                if sc + 2 < NSC:
                    ingest(sc + 2)
                ab = gemm_a(sc)
                xs_cur = scan(sc, ab, xs_prev)
                gemm_b(sc - 1, xs_prev)
                xs_prev = xs_cur
            gemm_b(NSC - 1, xs_prev)

    nc.compile()
    return nc


def _get_program():
    if "nc" not in _CACHE:
        _CACHE["nc"] = _build_program()
    return _CACHE["nc"]


def kernel(u, Wa, ba, Wb, bb, C, Dm):
    global LAST_RESULTS
    from concourse.bass_utils import run_bass_kernel_spmd

    nc = _get_program()

    u = np.asarray(u, dtype=np.float32)
    bf = ml_dtypes.bfloat16
    f8 = ml_dtypes.float8_e4m3
    wab = np.concatenate([np.asarray(Wa), np.asarray(Wb)], axis=0).T   # (D, 2N)
    wab8_np = np.ascontiguousarray(
        (np.asarray(wab, np.float32) * WAB_SCALE)
        .reshape(KT, 128, 2 * N).transpose(1, 0, 2)
    ).astype(f8)                                                       # (128, KT, 2N)
    bias_np = np.ascontiguousarray(
        np.concatenate([np.asarray(ba), np.asarray(bb)]).astype(np.float32)
        .reshape(4, 128).T
    )                                                                  # (128, 4)
    ct_np = np.ascontiguousarray(np.asarray(C).T).astype(bf)           # (N, D)
    dmt_np = np.ascontiguousarray(np.asarray(Dm).T).astype(bf)         # (D, D)

    in_maps = [
        {
            "u": np.ascontiguousarray(u[b]),
            "wab8": wab8_np,
            "bias": bias_np,
            "ct": ct_np,
            "dmt": dmt_np,
        }
        for b in range(B)
    ]

    res = run_bass_kernel_spmd(nc, in_maps, core_ids=list(range(NCORES)))
    LAST_RESULTS = res
    return np.stack([r["y"] for r in res.results], axis=0)
